# revision 16
# baseline (speedup 1.0000x reference)
"""Slot-attention kernel v8 for Trainium2, SPMD over 8 NeuronCores.

Reference computation (per batch element b):
  query[b,n,:] = q[n,b,:] @ qw[n]          (n = 32 query slots)
  keyp [b,m,:] = k[m,b,:] @ kw[m]          (m = 32 key slots)
  value[b,m,:] = k[m,b,:] @ vw[m]
  logits[b,n,m] = query[b,n,:]·keyp[b,m,:] / 16
  attn = softmax_m(logits)
  out[n,b,:] = sum_m attn[b,n,m] * value[b,m,:]

Sharding: data-parallel over batch (4096 -> 512 per core), weights replicated.

Design (selected over 10 measured variants):
  - single full-width phase A (batch 512 moving per matmul, weights loaded
    exactly once): halved-batch variants double weight DMA and halve matmul
    length, measuring consistently slower
  - all-bf16 compute (fp8 V costs 4.5e-2 rel err - value-quantization noise
    passes straight through the attention-weighted sum)
  - SBUF cannot hold QTs + KTs + V32Q at full width (3 x 64 KB), so V is
    spilled to a DRAM scratch in its shuffled [32*j+m, g, o] layout during
    phase A (gpsimd SWDGE queue, fully overlapped) and read back as fat
    contiguous per-quad chunks (8 KB runs) during phases B/C
  - inputs pre-tiled on host so every input DMA is contiguous (2-4 KB runs)
  - exp/reduce/transpose batched to [128, 512] per 16-group quad
  - phase C software-pipelined one quad behind phase B
  - psum evacuations at FD-512, alternating Vector/Scalar engines, 8-bank
    rotation (deeper psum rotation beat fewer/larger evac ops on hardware)
  - bf16 output, upcast on host
"""

import numpy as np
import ml_dtypes

import concourse.bass as bass
from concourse import bacc
import concourse.mybir as mybir
import concourse.tile as tile
from concourse.bass_utils import run_bass_kernel_spmd

BF16 = mybir.dt.bfloat16
F32 = mybir.dt.float32

NQ = 32          # query slots
NK = 32          # key slots
D = 256          # input dim (contraction of projections)
A = 256          # attn dim (contraction of logits)
O = 256          # out dim
BS = 4096
N_CORES = 8
BS_CORE = BS // N_CORES   # 512


def build_kernel(bs_core=BS_CORE):
    nc = bacc.Bacc()

    n_groups = bs_core // 4            # 4-batch groups (128)
    n_quads = n_groups // 16           # exp/transpose batches (8)
    jstride = n_groups                 # batch stride between j-blocks (128)

    # host-pre-tiled inputs: every DMA source is contiguous
    qTt = nc.declare_dram_parameter("qTt", [NQ, 128, 2, bs_core], BF16,
                                    isOutput=False)
    kTt = nc.declare_dram_parameter("kTt", [NQ, 128, 2, bs_core], BF16,
                                    isOutput=False)
    wqkt = nc.declare_dram_parameter("wqkt", [NQ, 128, 2, 2 * A], BF16,
                                     isOutput=False)
    wvt = nc.declare_dram_parameter("wvt", [NQ, 128, 2, O], BF16,
                                    isOutput=False)
    out = nc.declare_dram_parameter("out", [NQ, bs_core, O], BF16,
                                    isOutput=True)

    with tile.TileContext(nc) as tc:
        with (
            tc.tile_pool(name="xin", bufs=4) as xin,
            tc.tile_pool(name="win", bufs=3) as win,
            tc.tile_pool(name="big", bufs=1) as big,
            tc.tile_pool(name="v32", bufs=3) as v32p,
            tc.tile_pool(name="vnp", bufs=6) as vnp,
            tc.tile_pool(name="eqp", bufs=3) as eqp,
            tc.tile_pool(name="tep", bufs=3) as tep,
            tc.tile_pool(name="smp", bufs=3) as smp,
            tc.tile_pool(name="outp", bufs=2) as outp,
            tc.tile_pool(name="dsp", bufs=1, space="DRAM") as dsp,
            tc.tile_pool(name="ps", bufs=8, space="PSUM") as psp,
        ):
            # DRAM scratch for the shuffled V: [32*j + m, g, o]
            VSP = dsp.tile([128, n_groups, O], BF16, tag="VSP")

            QTs = big.tile([128, 2, NQ, bs_core], BF16, tag="QTs")
            KTs = big.tile([128, 2, NK, bs_core], BF16, tag="KTs")
            rs = big.tile([128, n_quads, 16], F32, tag="rs")

            # ---- Phase A: projections ----
            for s in range(NQ):
                qts = xin.tile([128, 2, bs_core], BF16, tag="qts")
                nc.sync.dma_start(out=qts, in_=qTt[s])
                kts = xin.tile([128, 2, bs_core], BF16, tag="kts")
                nc.sync.dma_start(out=kts, in_=kTt[s])
                wsg = win.tile([128, 2, 2 * A], BF16, tag="wsg")
                nc.sync.dma_start(out=wsg, in_=wqkt[s])
                wvs = win.tile([128, 2, O], BF16, tag="wvs")
                nc.sync.dma_start(out=wvs, in_=wvt[s])

                # Q / K projections: weight-stationary, N=512 moving
                for pi in range(2):
                    xs = qts if pi == 0 else kts
                    dst = QTs if pi == 0 else KTs
                    for t in range(2):  # a-tile
                        ps = psp.tile([128, bs_core], F32, tag="bank")
                        for c in range(2):
                            nc.tensor.matmul(
                                ps,
                                lhsT=wsg[:, c, pi * A + t * 128:
                                         pi * A + (t + 1) * 128],
                                rhs=xs[:, c, :],
                                start=(c == 0),
                                stop=(c == 1),
                            )
                        # evac psum -> sbuf (1/16 temperature folded into Q)
                        if pi == 0:
                            if t == 0:
                                nc.scalar.mul(dst[:, t, s, :], ps, 1.0 / 16.0)
                            else:
                                nc.vector.tensor_scalar_mul(
                                    out=dst[:, t, s, :], in0=ps,
                                    scalar1=1.0 / 16.0)
                        else:
                            if t == 0:
                                nc.scalar.copy(out=dst[:, t, s, :], in_=ps)
                            else:
                                nc.vector.tensor_copy(out=dst[:, t, s, :],
                                                      in_=ps)

                # V projection: x-stationary (psum partition = batch)
                for h2 in range(2):
                    vps = psp.tile([128, 2, O], F32, tag="bank")
                    for bi in range(2):
                        bb = h2 * 2 + bi
                        for c in range(2):
                            nc.tensor.matmul(
                                vps[:, bi, :],
                                lhsT=kts[:, c, bb * 128:(bb + 1) * 128],
                                rhs=wvs[:, c, :],
                                start=(c == 0),
                                stop=(c == 1),
                            )
                    vn = vnp.tile([128, 2, O], BF16, tag="vn")
                    if h2 == 0:
                        nc.scalar.copy(out=vn, in_=vps)
                    else:
                        nc.vector.tensor_copy(out=vn, in_=vps)
                    # spill shuffled: V[b = bb*128 + p][m=s, o] ->
                    #   VSP[32*bb + s, g=p, o]  (dram row is contiguous 64KB)
                    for bi in range(2):
                        bb = h2 * 2 + bi
                        row = 32 * bb + s
                        nc.gpsimd.dma_start(
                            out=VSP[row:row + 1, :, :],
                            in_=vn[:, bi, :],
                        )

            # ---- Phase B + pipelined C ----
            TEs = [None] * n_quads
            VCs = [None] * n_quads

            def phase_c(qd, TEq, VCq):
                for g8 in range(2):
                    OUTo = outp.tile([128, 8, O], BF16, tag="OUTo")
                    for g4 in range(4):
                        av = psp.tile([128, 2, O], F32, tag="bank")
                        for gi in range(2):
                            gq = g8 * 8 + g4 * 2 + gi
                            for j in range(4):
                                nc.tensor.matmul(
                                    av[32 * j:32 * (j + 1), gi, :],
                                    lhsT=TEq[32 * j:32 * (j + 1), gq, :],
                                    rhs=VCq[32 * j:32 * (j + 1), gq, :],
                                    start=True, stop=True,
                                    tile_position=(32 * j, 32 * j),
                                    skip_group_check=True,
                                )
                        for gi in range(2):
                            gq = g8 * 8 + g4 * 2 + gi
                            if gi == 0:
                                nc.scalar.mul(OUTo[:, g4 * 2 + gi, :],
                                              av[:, gi, :],
                                              rs[:, qd, gq:gq + 1])
                            else:
                                nc.vector.tensor_scalar_mul(
                                    out=OUTo[:, g4 * 2 + gi, :],
                                    in0=av[:, gi, :],
                                    scalar1=rs[:, qd, gq:gq + 1])
                    g0 = qd * 16 + g8 * 8
                    for j in range(4):
                        nc.sync.dma_start(
                            out=out[:, jstride * j + g0:
                                    jstride * j + g0 + 8, :],
                            in_=OUTo[32 * j:32 * (j + 1), :, :],
                        )

            for qd in range(n_quads):
                # prefetch this quad's V chunk back from the DRAM spill
                VCq = v32p.tile([128, 16, O], BF16, tag="VCq")
                nc.scalar.dma_start(out=VCq,
                                    in_=VSP[:, qd * 16:(qd + 1) * 16, :])
                VCs[qd] = VCq

                lg = psp.tile([128, 16, NK], F32, tag="bank")
                for gi in range(16):
                    g = qd * 16 + gi
                    for c in range(2):
                        for j in range(4):
                            b = g + jstride * j
                            nc.tensor.matmul(
                                lg[32 * j:32 * (j + 1), gi, :],
                                lhsT=QTs[:, c, :, b],
                                rhs=KTs[:, c, :, b],
                                start=(c == 0),
                                stop=(c == 1),
                                tile_position=(0, 32 * j),
                                skip_group_check=True,
                            )
                Eq = eqp.tile([128, 16, NK], BF16, tag="Eq")
                nc.scalar.activation(
                    out=Eq.rearrange("p a b -> p (a b)"),
                    in_=lg.rearrange("p a b -> p (a b)"),
                    func=mybir.ActivationFunctionType.Exp,
                )
                sm = smp.tile([128, 16], F32, tag="sm")
                nc.vector.reduce_sum(out=sm, in_=Eq,
                                     axis=mybir.AxisListType.X)
                nc.vector.reciprocal(out=rs[:, qd, :], in_=sm)
                TEq = tep.tile([128, 16, NQ], BF16, tag="TEq")
                nc.vector.transpose(out=TEq, in_=Eq)
                TEs[qd] = TEq
                if qd > 0:
                    phase_c(qd - 1, TEs[qd - 1], VCs[qd - 1])
            phase_c(n_quads - 1, TEs[n_quads - 1], VCs[n_quads - 1])
    return nc


def _tile_x(xT):
    # xT [32, 256, bs_core] -> [slot, p, c, b]
    return np.ascontiguousarray(
        xT.reshape(NQ, 2, 128, -1).transpose(0, 2, 1, 3))


def _prep_inputs(q, k, query_weight, key_weight, value_weight, bs_core):
    bf = ml_dtypes.bfloat16
    wqk = np.stack((query_weight, key_weight), axis=2).astype(bf)
    # [32, 256, 2, 256] -> [slot, p, c, (w a)]
    wqkt = np.ascontiguousarray(
        wqk.reshape(NQ, 2, 128, 2, A).transpose(0, 2, 1, 3, 4)
        .reshape(NQ, 128, 2, 2 * A))
    wvt = np.ascontiguousarray(
        value_weight.astype(bf).reshape(NQ, 2, 128, O)
        .transpose(0, 2, 1, 3))
    in_maps = []
    for i in range(N_CORES):
        sl = slice(i * bs_core, (i + 1) * bs_core)
        qTb = q[:, sl, :].transpose(0, 2, 1).astype(bf)
        kTb = k[:, sl, :].transpose(0, 2, 1).astype(bf)
        in_maps.append({"qTt": _tile_x(qTb), "kTt": _tile_x(kTb),
                        "wqkt": wqkt, "wvt": wvt})
    return in_maps


_NC_CACHE = {}


def _get_nc(bs_core):
    if bs_core not in _NC_CACHE:
        nc = build_kernel(bs_core)
        nc.finalize()
        _NC_CACHE[bs_core] = nc
    return _NC_CACHE[bs_core]


def kernel(q, k, query_weight, key_weight, value_weight, _trace=False):
    nc = _get_nc(BS_CORE)
    in_maps = _prep_inputs(q, k, query_weight, key_weight, value_weight,
                           BS_CORE)
    res = run_bass_kernel_spmd(nc, in_maps, core_ids=list(range(N_CORES)),
                               trace=_trace)
    outs = [res.results[i]["out"] for i in range(N_CORES)]
    full = np.concatenate(outs, axis=1).astype(np.float32)
    if _trace:
        return full, res
    return full


# revision 17
# speedup vs baseline: 1.1928x; 1.1928x over previous
"""Slot-attention kernel v8 for Trainium2, SPMD over 8 NeuronCores.

Reference computation (per batch element b):
  query[b,n,:] = q[n,b,:] @ qw[n]          (n = 32 query slots)
  keyp [b,m,:] = k[m,b,:] @ kw[m]          (m = 32 key slots)
  value[b,m,:] = k[m,b,:] @ vw[m]
  logits[b,n,m] = query[b,n,:]·keyp[b,m,:] / 16
  attn = softmax_m(logits)
  out[n,b,:] = sum_m attn[b,n,m] * value[b,m,:]

Sharding: data-parallel over batch (4096 -> 512 per core), weights replicated.

Design (selected over 10 measured variants):
  - single full-width phase A (batch 512 moving per matmul, weights loaded
    exactly once): halved-batch variants double weight DMA and halve matmul
    length, measuring consistently slower
  - all-bf16 compute (fp8 V costs 4.5e-2 rel err - value-quantization noise
    passes straight through the attention-weighted sum)
  - SBUF cannot hold QTs + KTs + V32Q at full width (3 x 64 KB), so V is
    spilled to a DRAM scratch in its shuffled [32*j+m, g, o] layout during
    phase A (gpsimd SWDGE queue, fully overlapped) and read back as fat
    contiguous per-quad chunks (8 KB runs) during phases B/C
  - inputs pre-tiled on host so every input DMA is contiguous (2-4 KB runs)
  - exp/reduce/transpose batched to [128, 512] per 16-group quad
  - phase C software-pipelined one quad behind phase B
  - psum evacuations at FD-512, alternating Vector/Scalar engines, 8-bank
    rotation (deeper psum rotation beat fewer/larger evac ops on hardware)
  - bf16 output, upcast on host
"""

import numpy as np
import ml_dtypes

import concourse.bass as bass
from concourse import bacc
import concourse.mybir as mybir
import concourse.tile as tile
from concourse.bass_utils import run_bass_kernel_spmd

BF16 = mybir.dt.bfloat16
F32 = mybir.dt.float32

NQ = 32          # query slots
NK = 32          # key slots
D = 256          # input dim (contraction of projections)
A = 256          # attn dim (contraction of logits)
O = 256          # out dim
BS = 4096
N_CORES = 8
BS_CORE = BS // N_CORES   # 512


def build_kernel(bs_core=BS_CORE):
    nc = bacc.Bacc()

    n_groups = bs_core // 4            # 4-batch groups (128)
    n_quads = n_groups // 16           # exp/transpose batches (8)
    jstride = n_groups                 # batch stride between j-blocks (128)

    # host-pre-tiled inputs: every DMA source is contiguous
    qTt = nc.declare_dram_parameter("qTt", [NQ, 128, 2, bs_core], BF16,
                                    isOutput=False)
    kTt = nc.declare_dram_parameter("kTt", [NQ, 128, 2, bs_core], BF16,
                                    isOutput=False)
    wqkt = nc.declare_dram_parameter("wqkt", [NQ, 128, 2, 2 * A], BF16,
                                     isOutput=False)
    wvt = nc.declare_dram_parameter("wvt", [NQ, 128, 2, O], BF16,
                                    isOutput=False)
    out = nc.declare_dram_parameter("out", [NQ, bs_core, O], BF16,
                                    isOutput=True)

    with tile.TileContext(nc) as tc:
        with (
            tc.tile_pool(name="xin", bufs=4) as xin,
            tc.tile_pool(name="win", bufs=3) as win,
            tc.tile_pool(name="big", bufs=1) as big,
            tc.tile_pool(name="v32", bufs=3) as v32p,
            tc.tile_pool(name="vnp", bufs=6) as vnp,
            tc.tile_pool(name="eqp", bufs=3) as eqp,
            tc.tile_pool(name="tep", bufs=3) as tep,
            tc.tile_pool(name="smp", bufs=3) as smp,
            tc.tile_pool(name="outp", bufs=2) as outp,
            tc.tile_pool(name="dsp", bufs=1, space="DRAM") as dsp,
            tc.tile_pool(name="ps", bufs=8, space="PSUM") as psp,
        ):
            # DRAM scratch for the shuffled V: [32*j + m, g, o]
            VSP = dsp.tile([128, n_groups, O], BF16, tag="VSP")

            QTs = big.tile([128, 2, NQ, bs_core], BF16, tag="QTs")
            KTs = big.tile([128, 2, NK, bs_core], BF16, tag="KTs")
            rs = big.tile([128, n_quads, 16], F32, tag="rs")

            # ---- Phase A: projections ----
            for s in range(NQ):
                qts = xin.tile([128, 2, bs_core], BF16, tag="qts")
                nc.sync.dma_start(out=qts, in_=qTt[s])
                kts = xin.tile([128, 2, bs_core], BF16, tag="kts")
                nc.sync.dma_start(out=kts, in_=kTt[s])
                wsg = win.tile([128, 2, 2 * A], BF16, tag="wsg")
                nc.sync.dma_start(out=wsg, in_=wqkt[s])
                wvs = win.tile([128, 2, O], BF16, tag="wvs")
                nc.sync.dma_start(out=wvs, in_=wvt[s])

                # Q / K projections: weight-stationary, N=512 moving
                for pi in range(2):
                    xs = qts if pi == 0 else kts
                    dst = QTs if pi == 0 else KTs
                    for t in range(2):  # a-tile
                        ps = psp.tile([128, bs_core], F32, tag="bank")
                        for c in range(2):
                            nc.tensor.matmul(
                                ps,
                                lhsT=wsg[:, c, pi * A + t * 128:
                                         pi * A + (t + 1) * 128],
                                rhs=xs[:, c, :],
                                start=(c == 0),
                                stop=(c == 1),
                            )
                        # evac psum -> sbuf (1/16 temperature folded into Q)
                        if pi == 0:
                            if t == 0:
                                nc.scalar.mul(dst[:, t, s, :], ps, 1.0 / 16.0)
                            else:
                                nc.vector.tensor_scalar_mul(
                                    out=dst[:, t, s, :], in0=ps,
                                    scalar1=1.0 / 16.0)
                        else:
                            if t == 0:
                                nc.scalar.copy(out=dst[:, t, s, :], in_=ps)
                            else:
                                nc.vector.tensor_copy(out=dst[:, t, s, :],
                                                      in_=ps)

                # V projection: x-stationary (psum partition = batch)
                for h2 in range(2):
                    vps = psp.tile([128, 2, O], F32, tag="bank")
                    for bi in range(2):
                        bb = h2 * 2 + bi
                        for c in range(2):
                            nc.tensor.matmul(
                                vps[:, bi, :],
                                lhsT=kts[:, c, bb * 128:(bb + 1) * 128],
                                rhs=wvs[:, c, :],
                                start=(c == 0),
                                stop=(c == 1),
                            )
                    vn = vnp.tile([128, 2, O], BF16, tag="vn")
                    if h2 == 0:
                        nc.scalar.copy(out=vn, in_=vps)
                    else:
                        nc.vector.tensor_copy(out=vn, in_=vps)
                    # spill shuffled: V[b = bb*128 + p][m=s, o] ->
                    #   VSP[32*bb + s, g=p, o]  (dram row is contiguous 64KB)
                    for bi in range(2):
                        bb = h2 * 2 + bi
                        row = 32 * bb + s
                        # split spill writes across two queues: a single
                        # SWDGE queue lags ~50us behind phase A and stalls
                        # phase C behind the last spill + readback
                        eng = nc.gpsimd if bi == 0 else nc.sync
                        eng.dma_start(
                            out=VSP[row:row + 1, :, :],
                            in_=vn[:, bi, :],
                        )

            # ---- Phase B + pipelined C ----
            TEs = [None] * n_quads
            VCs = [None] * n_quads

            def phase_c(qd, TEq, VCq):
                for g8 in range(2):
                    OUTo = outp.tile([128, 8, O], BF16, tag="OUTo")
                    for g4 in range(4):
                        av = psp.tile([128, 2, O], F32, tag="bank")
                        for gi in range(2):
                            gq = g8 * 8 + g4 * 2 + gi
                            for j in range(4):
                                nc.tensor.matmul(
                                    av[32 * j:32 * (j + 1), gi, :],
                                    lhsT=TEq[32 * j:32 * (j + 1), gq, :],
                                    rhs=VCq[32 * j:32 * (j + 1), gq, :],
                                    start=True, stop=True,
                                    tile_position=(32 * j, 32 * j),
                                    skip_group_check=True,
                                )
                        for gi in range(2):
                            gq = g8 * 8 + g4 * 2 + gi
                            if gi == 0:
                                nc.scalar.mul(OUTo[:, g4 * 2 + gi, :],
                                              av[:, gi, :],
                                              rs[:, qd, gq:gq + 1])
                            else:
                                nc.vector.tensor_scalar_mul(
                                    out=OUTo[:, g4 * 2 + gi, :],
                                    in0=av[:, gi, :],
                                    scalar1=rs[:, qd, gq:gq + 1])
                    g0 = qd * 16 + g8 * 8
                    for j in range(4):
                        nc.sync.dma_start(
                            out=out[:, jstride * j + g0:
                                    jstride * j + g0 + 8, :],
                            in_=OUTo[32 * j:32 * (j + 1), :, :],
                        )

            for qd in range(n_quads):
                # prefetch this quad's V chunk back from the DRAM spill
                VCq = v32p.tile([128, 16, O], BF16, tag="VCq")
                nc.scalar.dma_start(out=VCq,
                                    in_=VSP[:, qd * 16:(qd + 1) * 16, :])
                VCs[qd] = VCq

                lg = psp.tile([128, 16, NK], F32, tag="bank")
                for gi in range(16):
                    g = qd * 16 + gi
                    for c in range(2):
                        for j in range(4):
                            b = g + jstride * j
                            nc.tensor.matmul(
                                lg[32 * j:32 * (j + 1), gi, :],
                                lhsT=QTs[:, c, :, b],
                                rhs=KTs[:, c, :, b],
                                start=(c == 0),
                                stop=(c == 1),
                                tile_position=(0, 32 * j),
                                skip_group_check=True,
                            )
                Eq = eqp.tile([128, 16, NK], BF16, tag="Eq")
                nc.scalar.activation(
                    out=Eq.rearrange("p a b -> p (a b)"),
                    in_=lg.rearrange("p a b -> p (a b)"),
                    func=mybir.ActivationFunctionType.Exp,
                )
                sm = smp.tile([128, 16], F32, tag="sm")
                nc.vector.reduce_sum(out=sm, in_=Eq,
                                     axis=mybir.AxisListType.X)
                nc.vector.reciprocal(out=rs[:, qd, :], in_=sm)
                TEq = tep.tile([128, 16, NQ], BF16, tag="TEq")
                nc.vector.transpose(out=TEq, in_=Eq)
                TEs[qd] = TEq
                if qd > 0:
                    phase_c(qd - 1, TEs[qd - 1], VCs[qd - 1])
            phase_c(n_quads - 1, TEs[n_quads - 1], VCs[n_quads - 1])
    return nc


def _tile_x(xT):
    # xT [32, 256, bs_core] -> [slot, p, c, b]
    return np.ascontiguousarray(
        xT.reshape(NQ, 2, 128, -1).transpose(0, 2, 1, 3))


def _prep_inputs(q, k, query_weight, key_weight, value_weight, bs_core):
    bf = ml_dtypes.bfloat16
    wqk = np.stack((query_weight, key_weight), axis=2).astype(bf)
    # [32, 256, 2, 256] -> [slot, p, c, (w a)]
    wqkt = np.ascontiguousarray(
        wqk.reshape(NQ, 2, 128, 2, A).transpose(0, 2, 1, 3, 4)
        .reshape(NQ, 128, 2, 2 * A))
    wvt = np.ascontiguousarray(
        value_weight.astype(bf).reshape(NQ, 2, 128, O)
        .transpose(0, 2, 1, 3))
    in_maps = []
    for i in range(N_CORES):
        sl = slice(i * bs_core, (i + 1) * bs_core)
        qTb = q[:, sl, :].transpose(0, 2, 1).astype(bf)
        kTb = k[:, sl, :].transpose(0, 2, 1).astype(bf)
        in_maps.append({"qTt": _tile_x(qTb), "kTt": _tile_x(kTb),
                        "wqkt": wqkt, "wvt": wvt})
    return in_maps


_NC_CACHE = {}


def _get_nc(bs_core):
    if bs_core not in _NC_CACHE:
        nc = build_kernel(bs_core)
        nc.finalize()
        _NC_CACHE[bs_core] = nc
    return _NC_CACHE[bs_core]


def kernel(q, k, query_weight, key_weight, value_weight, _trace=False):
    nc = _get_nc(BS_CORE)
    in_maps = _prep_inputs(q, k, query_weight, key_weight, value_weight,
                           BS_CORE)
    res = run_bass_kernel_spmd(nc, in_maps, core_ids=list(range(N_CORES)),
                               trace=_trace)
    outs = [res.results[i]["out"] for i in range(N_CORES)]
    full = np.concatenate(outs, axis=1).astype(np.float32)
    if _trace:
        return full, res
    return full
